# revision 3
# baseline (speedup 1.0000x reference)
"""2-layer LSTM encoder (batch collapsed into recurrence) on TRN2 — v2.

Structure (single core; collectives are impractical per-step here):
  GEMM0: x_pre0 = seq @ W_ih0.T + b0   -> staged j-major in DRAM (bf16)
  rec0 : 4 segments of S/4 steps; x_pre0 segment streamed into SBUF;
         per step 576 LDW+MM pairs (48 gate-cols x 12 K-tiles, N=1) +
         9-op cell; h0 archived to SBUF (no HBM round trip).
  Per segment s: GEMM1(seg) computes x_pre1 = H0 @ W_ih1.T + b1 directly
         into SBUF (no DRAM staging), then rec1 over the segment.
  Weights are stationary in one 144KB/partition SBUF slot, reloaded per
  phase/segment (W_ih1 <-> W_hh1 swap per segment).

v1 emitted ~60k static instructions (24 chunk-unrolled loops); most of
the measured time was per-call host lowering proportional to program
size, not device time.  v2 is ~6k instructions with identical structure
at any S (so the small-S calibration run in test.py subtracts the host
overhead exactly), and avoids all per-step DRAM traffic.
"""

import sys

sys.path.insert(0, "/opt/trn_rl_repo")
import numpy as np
import ml_dtypes
import concourse.bass as bass
import concourse.bacc as bacc
import concourse.mybir as mybir
from concourse import tile
from concourse.tile_rust import add_dep_helper
from contextlib import ExitStack

F32 = mybir.dt.float32
BF16 = mybir.dt.bfloat16
AF = mybir.ActivationFunctionType

B, T, D, H = 16, 64, 256, 1536
NB = H // 128           # 12 unit blocks
NJ = 4 * NB             # 48 psum cols
NK = H // 128           # 12 K-tiles (H contraction)
KD = D // 128           # 2 K-tiles (D contraction)
SEG = 4                 # recurrence segments (x_pre SBUF-resident per seg)

_IOFF, _FOFF, _GOFF, _OOFF = 0, H, 2 * H, 3 * H


def col_gate(j):
    if j < 36:
        return [_IOFF, _FOFF, _OOFF][j % 3], j // 3
    return _GOFF, j - 36


def gate_rows(j):
    goff, blk = col_gate(j)
    return np.arange(goff + 128 * blk, goff + 128 * blk + 128)


def pack_lhsT(W, nk):
    out = np.zeros((128, nk * NJ * 128), dtype=W.dtype)
    for k in range(nk):
        for j in range(NJ):
            out[:, (k * NJ + j) * 128 : (k * NJ + j + 1) * 128] = W[
                gate_rows(j), 128 * k : 128 * (k + 1)
            ].T
    return out


def pack_biasT(b):
    out = np.zeros((128, NJ), dtype=np.float32)
    for j in range(NJ):
        out[:, j] = b[gate_rows(j)]
    return out


def prep_inputs(batch, W_ih0, W_hh0, b_ih0, b_hh0, W_ih1, W_hh1, b_ih1, b_hh1,
                S=None):
    bf = ml_dtypes.bfloat16
    seq = np.ascontiguousarray(
        np.asarray(batch)[:, 1:, :].transpose(1, 0, 2).reshape(-1, D)
    ).astype(np.float32)
    if S is not None:
        seq = seq[:S]
    S = seq.shape[0]
    seqt = np.ascontiguousarray(seq.T)
    b0 = (np.asarray(b_ih0) + np.asarray(b_hh0)).astype(np.float32)
    b1 = (np.asarray(b_ih1) + np.asarray(b_hh1)).astype(np.float32)
    m = {
        "seqt": np.ascontiguousarray(
            seqt.reshape(KD, 128, S).transpose(1, 0, 2).reshape(128, KD * S)
        ).astype(bf),
        "wih0t": pack_lhsT(np.asarray(W_ih0).astype(bf), KD),
        "whh0t": pack_lhsT(np.asarray(W_hh0).astype(bf), NK),
        "wih1t": pack_lhsT(np.asarray(W_ih1).astype(bf), NK),
        "whh1t": pack_lhsT(np.asarray(W_hh1).astype(bf), NK),
        "b0t": pack_biasT(b0),
        "b1t": pack_biasT(b1),
    }
    return m, S


def build(S=1008, **_ignored):
    assert S % SEG == 0
    LS = S // SEG

    nc = bacc.Bacc(
        "TRN2",
        target_bir_lowering=False,
        debug=False,
        detect_race_conditions=False,
        num_devices=1,
    )

    seqt_e = nc.declare_dram_parameter("seqt", [128, KD * S], BF16, isOutput=False)
    wih0t_e = nc.declare_dram_parameter("wih0t", [128, KD * NJ * 128], BF16, isOutput=False)
    whh0t_e = nc.declare_dram_parameter("whh0t", [128, NK * NJ * 128], BF16, isOutput=False)
    wih1t_e = nc.declare_dram_parameter("wih1t", [128, NK * NJ * 128], BF16, isOutput=False)
    whh1t_e = nc.declare_dram_parameter("whh1t", [128, NK * NJ * 128], BF16, isOutput=False)
    b0t_e = nc.declare_dram_parameter("b0t", [128, NJ], F32, isOutput=False)
    b1t_e = nc.declare_dram_parameter("b1t", [128, NJ], F32, isOutput=False)
    hc_e = nc.declare_dram_parameter("hc", [128, 4 * NB], F32, isOutput=True)

    # x_pre0 staging in DRAM, j-major: col j*S + t (bf16)
    xp0_d = nc.dram_tensor("xp0d", [128, NJ * S], BF16)

    with tile.TileContext(nc) as tc, ExitStack() as ctx:
        pool = ctx.enter_context(tc.tile_pool(name="main", bufs=1))
        gsp = ctx.enter_context(tc.tile_pool(name="gst", bufs=2))
        pp = ctx.enter_context(tc.tile_pool(name="ps", bufs=2, space="PSUM"))
        gp = ctx.enter_context(tc.tile_pool(name="gps", bufs=2, space="PSUM"))

        bigw = pool.tile([128, NK * NJ * 128], BF16, tag="bigw")   # 144KB
        seqt = pool.tile([128, KD * S], BF16, tag="seqt")
        arch = pool.tile([128, S * NB], BF16, tag="arch")          # H0 archive
        b0t = pool.tile([128, NJ], F32, tag="b0t")
        b1t = pool.tile([128, NJ], F32, tag="b1t")
        cst = pool.tile([128, NB], F32, tag="cst")
        hfin = pool.tile([128, 4 * NB], F32, tag="hfin")
        hbf = pool.tile([128, NB], BF16, tag="hbf")
        gates = pool.tile([128, NJ], F32, tag="gates")
        sig = pool.tile([128, 36], F32, tag="sig")
        gt = pool.tile([128, NB], F32, tag="gt")
        th = pool.tile([128, NB], F32, tag="th")
        t1 = pool.tile([128, NB], F32, tag="t1")
        t2 = pool.tile([128, NB], F32, tag="t2")
        hf = pool.tile([128, NB], F32, tag="hf")

        nc.sync.dma_start(seqt[:], seqt_e[:])
        nc.sync.dma_start(b0t[:], b0t_e[:])
        nc.sync.dma_start(b1t[:], b1t_e[:])
        nc.vector.memset(hbf[:], 0.0)
        nc.vector.memset(cst[:], 0.0)
        # warm the activation table (sigmoid_and_others holds both fns) so
        # the in-loop activations need no table load
        nc.scalar.activation(t1[:], cst[:], AF.Sigmoid)
        nc.scalar.activation(t2[:], cst[:], AF.Tanh)

        seqt_r = seqt.rearrange("p (k t) -> p k t", k=KD)
        arch_r = arch.rearrange("p (t k) -> p t k", k=NB)
        xpd_r = xp0_d.rearrange("p (j t) -> p j t", j=NJ)

        # --- GEMM0: x_pre0 = seq @ W_ih0.T + b0 -> DRAM (j-major) ---
        nc.sync.dma_start(bigw[:, 0 : KD * NJ * 128], wih0t_e[:])
        for sg in range(SEG):
            t0 = sg * LS
            for j in range(NJ):
                gps = gp.tile([128, LS], F32, tag="gps", name="gps")
                for k in range(KD):
                    nc.tensor.matmul(
                        gps[:],
                        bigw[:, (k * NJ + j) * 128 : (k * NJ + j + 1) * 128],
                        seqt_r[:, k, t0 : t0 + LS],
                        start=(k == 0), stop=(k == KD - 1),
                    )
                gstg = gsp.tile([128, LS], BF16, tag="gstg", name="gstg")
                nc.vector.tensor_scalar_add(gstg[:], gps[:], b0t[:, j : j + 1])
                nc.sync.dma_start(
                    xp0_d[:, j * S + t0 : j * S + t0 + LS], gstg[:]
                )

        xps = pool.tile([128, NJ * LS], BF16, tag="xslot")
        xps_r = xps.rearrange("p (j t) -> p j t", j=NJ)

        def cell(xs_ap):
            nc.vector.tensor_add(gates[:], xs_ap, gates_ps[:])
            nc.scalar.activation(sig[:], gates[:, 0:36], AF.Sigmoid)
            nc.scalar.activation(gt[:], gates[:, 36:48], AF.Tanh)
            nc.vector.tensor_mul(t1[:], sig[:, 0:36:3], gt[:])
            nc.vector.tensor_mul(t2[:], sig[:, 1:36:3], cst[:])
            nc.vector.tensor_add(cst[:], t1[:], t2[:])
            nc.scalar.activation(th[:], cst[:], AF.Tanh)
            nc.vector.tensor_mul(hf[:], sig[:, 2:36:3], th[:])
            return nc.vector.tensor_copy(hbf[:], hf[:])

        # --- rec0: 4 segments, x_pre0 streamed to SBUF, h0 -> arch ---
        nc.sync.dma_start(bigw[:], whh0t_e[:])
        for sg in range(SEG):
            t0 = sg * LS
            nc.sync.dma_start(xps[:], xpd_r[:, :, t0 : t0 + LS])
            with tc.For_i(0, LS, hint_engines=(mybir.EngineType.PE,),
                          name=f"recA{sg}") as tt:
                gates_ps = pp.tile([128, NJ], F32, tag="mv", name="mv")
                for j in range(NJ):
                    for k in range(NK):
                        nc.tensor.matmul(
                            gates_ps[:, j : j + 1],
                            bigw[:, (k * NJ + j) * 128 : (k * NJ + j + 1) * 128],
                            hbf[:, k : k + 1],
                            start=(k == 0), stop=(k == NK - 1),
                        )
                xs = xps_r[:, :, bass.ds(tt, 1)].rearrange("p j one -> p (j one)")
                cell(xs)
                nc.gpsimd.tensor_copy(
                    arch[:, bass.ds(t0 * NB + tt * NB, NB)], hbf[:]
                )

        sv0 = nc.vector.tensor_copy(hfin[:, 0:NB], hf[:])
        sv1 = nc.vector.tensor_copy(hfin[:, NB : 2 * NB], cst[:])
        rst = nc.vector.memset(cst[:], 0.0)
        add_dep_helper(rst.ins, sv1.ins, reason="after save")
        rsh = nc.vector.memset(hbf[:], 0.0)

        # --- layer 1: per segment GEMM1 (SBUF-resident x_pre1) + rec1 ---
        for sg in range(SEG):
            t0 = sg * LS
            nc.sync.dma_start(bigw[:], wih1t_e[:])
            for j in range(NJ):
                gps = gp.tile([128, LS], F32, tag="gps", name="gps")
                for k in range(NK):
                    nc.tensor.matmul(
                        gps[:],
                        bigw[:, (k * NJ + j) * 128 : (k * NJ + j + 1) * 128],
                        arch_r[:, t0 : t0 + LS, k],
                        start=(k == 0), stop=(k == NK - 1),
                    )
                nc.vector.tensor_scalar_add(
                    xps[:, j * LS : (j + 1) * LS], gps[:], b1t[:, j : j + 1]
                )
            nc.sync.dma_start(bigw[:], whh1t_e[:])
            with tc.For_i(0, LS, hint_engines=(mybir.EngineType.PE,),
                          name=f"recB{sg}") as tt:
                gates_ps = pp.tile([128, NJ], F32, tag="mv", name="mv")
                for j in range(NJ):
                    for k in range(NK):
                        nc.tensor.matmul(
                            gates_ps[:, j : j + 1],
                            bigw[:, (k * NJ + j) * 128 : (k * NJ + j + 1) * 128],
                            hbf[:, k : k + 1],
                            start=(k == 0), stop=(k == NK - 1),
                        )
                xs = xps_r[:, :, bass.ds(tt, 1)].rearrange("p j one -> p (j one)")
                cell(xs)

        nc.vector.tensor_copy(hfin[:, 2 * NB : 3 * NB], hf[:])
        nc.vector.tensor_copy(hfin[:, 3 * NB : 4 * NB], cst[:])
        nc.sync.dma_start(hc_e[:], hfin[:])

    return nc


def assemble(results):
    h = np.zeros((2, H), np.float32)
    c = np.zeros((2, H), np.float32)
    hc = np.asarray(results[0]["hc"], dtype=np.float32)
    for blk in range(NB):
        u = 128 * blk
        h[0, u : u + 128] = hc[:, blk]
        c[0, u : u + 128] = hc[:, NB + blk]
        h[1, u : u + 128] = hc[:, 2 * NB + blk]
        c[1, u : u + 128] = hc[:, 3 * NB + blk]
    return h, c


def kernel(**inputs):
    """Full-input entry: build + compile + run on TRN2, return (h, c)."""
    from concourse.bass_utils import run_bass_kernel_spmd

    m, S = prep_inputs(**inputs)
    nc = build(S=S)
    nc.finalize()
    res = run_bass_kernel_spmd(nc, [m], [0])
    h, c = assemble(res.results)
    return h, c


if __name__ == "__main__":
    pass

